# revision 15
# baseline (speedup 1.0000x reference)
"""ButterflyLinear Trainium2 kernel.

Math insight: every one of the 12 butterfly stages pairs features strictly
within aligned groups of 4 (stage 0 pairs (4k,4k+1),(4k+2,4k+3); stages 1..11
all pair (4k,4k+2),(4k+1,4k+3)).  The whole network therefore collapses
exactly to a block-diagonal linear map with 1024 independent 4x4 blocks:

    out[t, 4k+j] = sum_i x[t, 4k+i] * M_k[i, j] + bias[4k+j]

M is extracted on the host (float64) by pushing the 4 group-basis vectors
through the stage chain.  The device kernel is a feature-major matmul pass:
the host ships x pre-transposed (feature-major tiles, contiguous rows),
each 128-feature chunk is one stationary-weight matmul
out_c[of, tok] = W_c.T @ x_c[if, tok] with N=512 tokens moving, bias added
per-partition during the PSUM->SBUF copy, and the host un-transposes the
returned output.  No on-device transposes, no identity, no bias broadcast.

x and out travel as fp16 (quantization rel err ~9e-4, well inside the 2e-2
gate), halving HBM traffic vs fp32 — this kernel is HBM-bound, so bytes
moved IS the runtime.  Weights are built on-device in fp32 and cast to fp16
for the PE's 2x 16-bit path; PSUM accumulation stays fp32.

Sharding: data-parallel over tokens, 8192/8 = 1024 tokens per core.
"""

import numpy as np

TOKENS = 8192
N = 4096
DEPTH = 12
NCORES = 8
TOK_PER_CORE = TOKENS // NCORES  # 1024
P = 128                  # partitions
N_CHUNKS = N // P        # 32 feature chunks of 128
GROUP = 4                # chunks per x/out group tile (4*1024 tok = 16KB rows)
N_GROUPS = N_CHUNKS // GROUP   # 8
TBLK = 512               # moving-token block per matmul (fp32 N<=512)
N_TBLK = TOK_PER_CORE // TBLK  # 2


def _apply_stage_np(x, factor, stage):
    B, n = x.shape
    block = 1 << (stage + 1)
    half = block >> 1
    m = n // block
    staged = x.reshape(B, m, half, 2).transpose(0, 1, 3, 2)
    pairs = staged.reshape(B, n // 2, 2)
    t = np.einsum("bnc,ncd->bnd", pairs, factor)
    t = t.reshape(B, m, 2, half).transpose(0, 1, 3, 2)
    return t.reshape(B, n)


def _compose_weights(factors):
    """Return M_cols [4, N] float64: M_cols[i, m] = Mfull[4*(m//4)+i, m]."""
    V = np.zeros((4, N), dtype=np.float64)
    for i in range(4):
        V[i, i::4] = 1.0
    M = V
    f64 = np.asarray(factors, dtype=np.float64)
    for s in range(DEPTH):
        M = _apply_stage_np(M, f64[s], s)
    return M


_PROG = None


def _get_program():
    global _PROG
    if _PROG is not None:
        return _PROG

    import concourse.mybir as mybir
    import concourse.tile as tile
    from concourse import bacc

    nc = bacc.Bacc("TRN2", target_bir_lowering=False, debug=False,
                   num_devices=NCORES)
    f32 = mybir.dt.float32
    f16 = mybir.dt.float16
    xp_h = nc.dram_tensor("xp", [N_GROUPS, P, GROUP * TOK_PER_CORE], f16,
                          kind="ExternalInput")
    m4_h = nc.dram_tensor("m4", [4, N], f32, kind="ExternalInput")
    sel_h = nc.dram_tensor("sel", [4, P], f32, kind="ExternalInput")
    msk_h = nc.dram_tensor("msk", [P, 2 * P], f32, kind="ExternalInput")
    bt_h = nc.dram_tensor("biast", [P, N_CHUNKS], f32, kind="ExternalInput")
    op_h = nc.dram_tensor("outp", [N_GROUPS, P, GROUP * TOK_PER_CORE], f16,
                          kind="ExternalOutput")

    xp = xp_h.ap()
    op = op_h.ap()

    HGRP = GROUP // 2          # 2 chunks per half-group unit
    HCOLS = HGRP * TOK_PER_CORE  # 2048 columns per unit

    with tile.TileContext(nc) as tc:
        with (
            tc.tile_pool(name="singles", bufs=1) as singles,
            tc.tile_pool(name="xin", bufs=6) as xpool,
            tc.tile_pool(name="oout", bufs=4) as opool,
            tc.tile_pool(name="ps", bufs=3, space="PSUM") as pspool,
            tc.tile_pool(name="wps", bufs=2, space="PSUM") as wpspool,
        ):
            bias_sb = singles.tile([P, N_CHUNKS], f32)
            nc.gpsimd.dma_start(out=bias_sb, in_=bt_h.ap())
            # Stationary weights are built on-device from ~160KB of compact
            # data: chunk c = (sel.T @ m4[:, c-slice]) * msk.  All small
            # inputs ride SWDGE (gpsimd) rings, which empirically execute
            # during the NRT preamble -- the sync HWDGE ring's first issue
            # is then the first x-tile load.
            m4_sb = singles.tile([4, N], f32)
            nc.gpsimd.dma_start(out=m4_sb, in_=m4_h.ap())
            sel_sb = singles.tile([4, P], f32)
            nc.gpsimd.dma_start(out=sel_sb, in_=sel_h.ap())
            msk_sb = singles.tile([P, 2 * P], f32)
            nc.gpsimd.dma_start(out=msk_sb, in_=msk_h.ap())
            w_sb = singles.tile([P, N], f16)

            # Units stream loads on nc.sync and stores on nc.scalar, so a
            # store waiting for compute never stalls the next load behind
            # it in the same engine queue.  The last group runs at quarter
            # granularity so the load->store pipeline latency at the tail
            # is halved.  Each unit covers `unit_chunks` feature chunks
            # (1024 tokens per chunk).
            units = [(g * GROUP + h * HGRP, HGRP) for g in range(N_GROUPS - 1)
                     for h in range(2)]
            units += [((N_GROUPS - 1) * GROUP + q, 1) for q in range(GROUP)]

            def build_weights(c0, nch):
                # One masked-multiply drains both build matmuls (fixed DVE
                # op cost amortized over 2 chunks).
                wp = wpspool.tile([P, 2 * P], f32, tag="wp")
                for cc in range(nch):
                    c = c0 + cc
                    nc.tensor.matmul(wp[:, cc * P:(cc + 1) * P], lhsT=sel_sb,
                                     rhs=m4_sb[:, c * P:(c + 1) * P],
                                     start=True, stop=True)
                nc.vector.tensor_mul(
                    w_sb[:, c0 * P:(c0 + nch) * P],
                    wp[:, 0:nch * P], msk_sb[:, 0:nch * P])

            # Weights are built one unit AHEAD so a unit's main matmuls
            # never wait on the (busy) DVE finishing this unit's mask-mul.
            build_weights(*units[0])

            pending_store = None
            for ui, (c0, nch) in enumerate(units):
                cols = nch * TOK_PER_CORE
                xg = xpool.tile([P, HCOLS], f16, tag="xg")
                nc.sync.dma_start(
                    out=xg[:, 0:cols],
                    in_=xp[c0 // GROUP, :,
                           (c0 % GROUP) * TOK_PER_CORE:
                           (c0 % GROUP) * TOK_PER_CORE + cols])
                if ui + 1 < len(units):
                    build_weights(*units[ui + 1])
                og = opool.tile([P, HCOLS], f16, tag="og")
                for cc in range(nch):
                    c = c0 + cc
                    # One 2-bank PSUM tile per chunk: both token-block
                    # matmuls land in it, one FD=1024 op drains it.
                    ps = pspool.tile([P, TOK_PER_CORE], f32, tag="ps")
                    for tb in range(N_TBLK):
                        nc.tensor.matmul(
                            ps[:, tb * TBLK:(tb + 1) * TBLK],
                            lhsT=w_sb[:, c * P:(c + 1) * P],
                            rhs=xg[:, cc * TOK_PER_CORE + tb * TBLK:
                                   cc * TOK_PER_CORE + (tb + 1) * TBLK],
                            start=True, stop=True,
                        )
                    dst = og[:, cc * TOK_PER_CORE:(cc + 1) * TOK_PER_CORE]
                    bcol = bias_sb[:, c:c + 1]
                    # PSUM->SBUF drains run in slow 1x mode (PSUM source),
                    # so they are the scarce resource: the two chunks of a
                    # unit always drain on DIFFERENT engines (ACT + DVE)
                    # so they overlap; 1-chunk tail units alternate.
                    on_act = (cc == 0) if nch == 2 else (c % 2 == 0)
                    if on_act:
                        nc.scalar.add(dst, ps, bcol)
                    else:
                        nc.vector.tensor_scalar_add(dst, ps, bcol)
                # Stores are emitted one unit LATE: by then both drains of
                # the stored unit are long done, so the ACT sequencer's
                # store issue never waits on a DVE semaphore.
                if pending_store is not None:
                    nc.scalar.dma_start(**pending_store)
                pending_store = dict(
                    out=op[c0 // GROUP, :,
                           (c0 % GROUP) * TOK_PER_CORE:
                           (c0 % GROUP) * TOK_PER_CORE + cols],
                    in_=og[:, 0:cols])
            nc.scalar.dma_start(**pending_store)

    nc.compile()
    _PROG = nc
    return nc


def _prep_core_input(xs):
    """[1024, 4096] fp16 token-major -> [8, 128, 4096] feature-major tiles.

    xprep[g, p, cc*1024 + t] = xs[t, (4g+cc)*128 + p]
    """
    xt = xs.T.reshape(N_GROUPS, GROUP, P, TOK_PER_CORE)   # [g][cc][p][t]
    return np.ascontiguousarray(
        xt.transpose(0, 2, 1, 3).reshape(N_GROUPS, P, GROUP * TOK_PER_CORE))


def _unprep_core_output(outp):
    """Inverse of _prep_core_input; fp16 device output -> fp32 token-major."""
    o = outp.reshape(N_GROUPS, P, GROUP, TOK_PER_CORE).transpose(0, 2, 1, 3)
    return o.reshape(N, TOK_PER_CORE).T.astype(np.float32)


def kernel(x, factors, bias):
    from concourse.bass_utils import run_bass_kernel_spmd

    x = np.asarray(x, dtype=np.float32)
    factors = np.asarray(factors, dtype=np.float32)
    bias_np = np.asarray(bias, dtype=np.float32)
    assert x.shape == (TOKENS, N)

    m4 = np.ascontiguousarray(_compose_weights(factors).astype(np.float32))
    pidx = np.arange(P)
    sel = np.ascontiguousarray(
        (pidx[None, :] % 4 == np.arange(4)[:, None]).astype(np.float32))
    msk1 = ((pidx[:, None] // 4) == (pidx[None, :] // 4)).astype(np.float32)
    msk = np.ascontiguousarray(np.tile(msk1, (1, 2)))
    biast = np.ascontiguousarray(bias_np.reshape(N_CHUNKS, P).T)

    nc = _get_program()
    x16 = x.astype(np.float16)
    in_maps = []
    for c in range(NCORES):
        in_maps.append({
            "xp": _prep_core_input(
                x16[c * TOK_PER_CORE:(c + 1) * TOK_PER_CORE]),
            "m4": m4,
            "sel": sel,
            "msk": msk,
            "biast": biast,
        })
    res = run_bass_kernel_spmd(nc, in_maps, core_ids=list(range(NCORES)))
    out = np.empty((TOKENS, N), dtype=np.float32)
    for c in range(NCORES):
        out[c * TOK_PER_CORE:(c + 1) * TOK_PER_CORE] = _unprep_core_output(
            res.results[c]["outp"])
    return out



# revision 36
# speedup vs baseline: 1.0614x; 1.0614x over previous
"""ButterflyLinear Trainium2 kernel.

Math insight: every one of the 12 butterfly stages pairs features strictly
within aligned groups of 4 (stage 0 pairs (4k,4k+1),(4k+2,4k+3); stages 1..11
all pair (4k,4k+2),(4k+1,4k+3)).  The whole network therefore collapses
exactly to a block-diagonal linear map with 1024 independent 4x4 blocks:

    out[t, 4k+j] = sum_i x[t, 4k+i] * M_k[i, j] + bias[4k+j]

M is extracted on the host (float64) by pushing the 4 group-basis vectors
through the stage chain.  The device kernel is a feature-major matmul pass:
the host ships x pre-transposed (feature-major tiles, contiguous rows),
each 128-feature chunk is one stationary-weight matmul
out_c[of, tok] = W_c.T @ x_c[if, tok] with N=512 tokens moving, bias added
per-partition during the PSUM->SBUF copy, and the host un-transposes the
returned output.  No on-device transposes, no identity, no bias broadcast.

x and out travel as fp16 (quantization rel err ~9e-4, well inside the 2e-2
gate), halving HBM traffic vs fp32 — this kernel is HBM-bound, so bytes
moved IS the runtime.  Weights are built on-device in fp32 and cast to fp16
for the PE's 2x 16-bit path; PSUM accumulation stays fp32.

Sharding: data-parallel over tokens, 8192/8 = 1024 tokens per core.
"""

import numpy as np

TOKENS = 8192
N = 4096
DEPTH = 12
NCORES = 8
TOK_PER_CORE = TOKENS // NCORES  # 1024
P = 128                  # partitions
N_CHUNKS = N // P        # 32 feature chunks of 128
GROUP = 4                # chunks per x/out group tile (4*1024 tok = 16KB rows)
N_GROUPS = N_CHUNKS // GROUP   # 8
TBLK = 512               # moving-token block per matmul (fp32 N<=512)
N_TBLK = TOK_PER_CORE // TBLK  # 2


def _apply_stage_np(x, factor, stage):
    B, n = x.shape
    block = 1 << (stage + 1)
    half = block >> 1
    m = n // block
    staged = x.reshape(B, m, half, 2).transpose(0, 1, 3, 2)
    pairs = staged.reshape(B, n // 2, 2)
    t = np.einsum("bnc,ncd->bnd", pairs, factor)
    t = t.reshape(B, m, 2, half).transpose(0, 1, 3, 2)
    return t.reshape(B, n)


def _compose_weights(factors):
    """Return M_cols [4, N] float64: M_cols[i, m] = Mfull[4*(m//4)+i, m]."""
    V = np.zeros((4, N), dtype=np.float64)
    for i in range(4):
        V[i, i::4] = 1.0
    M = V
    f64 = np.asarray(factors, dtype=np.float64)
    for s in range(DEPTH):
        M = _apply_stage_np(M, f64[s], s)
    return M


_PROG = None


def _get_program():
    global _PROG
    if _PROG is not None:
        return _PROG

    import concourse.mybir as mybir
    import concourse.tile as tile
    from concourse import bacc

    nc = bacc.Bacc("TRN2", target_bir_lowering=False, debug=False,
                   num_devices=NCORES)
    f32 = mybir.dt.float32
    f16 = mybir.dt.float16
    xp_h = nc.dram_tensor("xp", [N_GROUPS, P, GROUP * TOK_PER_CORE], f16,
                          kind="ExternalInput")
    wt_h = nc.dram_tensor("wt", [P, N], f16, kind="ExternalInput")
    bt_h = nc.dram_tensor("biast", [P, N_CHUNKS], f32, kind="ExternalInput")
    op_h = nc.dram_tensor("outp", [N_GROUPS, P, GROUP * TOK_PER_CORE], f16,
                          kind="ExternalOutput")

    xp = xp_h.ap()
    op = op_h.ap()

    with tile.TileContext(nc) as tc:
        with (
            tc.tile_pool(name="singles", bufs=1) as singles,
            tc.tile_pool(name="xin", bufs=3) as xpool,
            tc.tile_pool(name="oout", bufs=3) as opool,
            tc.tile_pool(name="ps", bufs=4, space="PSUM") as pspool,
        ):
            bias_sb = singles.tile([P, N_CHUNKS], f32)
            nc.gpsimd.dma_start(out=bias_sb, in_=bt_h.ap())
            # Stationary weights come pre-masked from the host (1MB fp16).
            # Every 128-row DMA costs ~2.6us of descriptor generation, so
            # both halves ride the store ring (idle until ~14us): their
            # generation runs in PARALLEL with the first x loads on sync,
            # and the first matmul starts ~4us earlier than if W led sync.
            w_sb = singles.tile([P, N], f16)
            nc.scalar.dma_start(out=w_sb[:, 0:N // 2], in_=wt_h.ap()[:, 0:N // 2])
            nc.scalar.dma_start(out=w_sb[:, N // 2:N], in_=wt_h.ap()[:, N // 2:N])

            # DMA rings retire ~1 descriptor / 20ns regardless of size and
            # descriptors are partition-row-sized, so ring bandwidth is
            # proportional to the contiguous row length.  Whole-group
            # transfers (4 chunks = 8KB fp16 rows, 1MB per DMA) keep both
            # rings HBM-bound instead of descriptor-bound; they also give
            # the PE long uninterrupted matmul runs so HAM warms to the
            # 2.4GHz clock.  Stores taper (2,1,1) at the end to shorten
            # the drain->last-store tail.
            # Group 0 loads in two halves so the first matmuls start one
            # descriptor-generation quantum (~2.8us) earlier.
            load_units = [(0, 2), (2, 2)]
            load_units += [(g * GROUP, GROUP) for g in range(1, N_GROUPS)]
            # First store unit is half-size so the store stream opens as
            # soon as the first two chunks drain.
            store_units = [(0, 2), (2, 2)]
            store_units += [(g * GROUP, GROUP) for g in range(1, N_GROUPS - 1)]
            store_units += [((N_GROUPS - 1) * GROUP, 2),
                            ((N_GROUPS - 1) * GROUP + 2, 1),
                            ((N_GROUPS - 1) * GROUP + 3, 1)]
            load_at = {c0: n for c0, n in load_units}
            store_of = {}
            for c0, n in store_units:
                for cc in range(n):
                    store_of[c0 + cc] = (c0, n, cc == n - 1)

            xg = og = None
            lu0 = su0 = 0
            for c in range(N_CHUNKS):
                if c in load_at:
                    lu0 = c
                    ln = load_at[c]
                    xg = xpool.tile([P, GROUP * TOK_PER_CORE], f16, tag="xg")
                    nc.sync.dma_start(
                        out=xg[:, 0:ln * TOK_PER_CORE],
                        in_=xp[c // GROUP, :,
                               (c % GROUP) * TOK_PER_CORE:
                               (c % GROUP + ln) * TOK_PER_CORE])
                su0, snch, closes = store_of[c]
                if c == su0:
                    og = opool.tile([P, GROUP * TOK_PER_CORE], f16, tag="og")
                # One 2-bank PSUM tile per chunk: both token-block matmuls
                # land in it, one FD=1024 op drains it.
                ps = pspool.tile([P, TOK_PER_CORE], f32, tag="ps")
                for tb in range(N_TBLK):
                    nc.tensor.matmul(
                        ps[:, tb * TBLK:(tb + 1) * TBLK],
                        lhsT=w_sb[:, c * P:(c + 1) * P],
                        rhs=xg[:, (c - lu0) * TOK_PER_CORE + tb * TBLK:
                               (c - lu0) * TOK_PER_CORE + (tb + 1) * TBLK],
                        start=True, stop=True,
                    )
                bcol = bias_sb[:, c:c + 1]
                # PSUM->SBUF drains run in slow 1x mode (PSUM source), so
                # they are the scarce resource: EVERY chunk's two PSUM
                # banks drain concurrently, ACT taking one and DVE the
                # other, halving per-chunk drain latency.
                o0 = (c - su0) * TOK_PER_CORE
                nc.scalar.add(og[:, o0:o0 + TBLK], ps[:, 0:TBLK], bcol)
                nc.vector.tensor_scalar_add(
                    og[:, o0 + TBLK:o0 + 2 * TBLK], ps[:, TBLK:2 * TBLK],
                    bcol)
                if closes:
                    cols = snch * TOK_PER_CORE
                    nc.scalar.dma_start(
                        out=op[su0 // GROUP, :,
                               (su0 % GROUP) * TOK_PER_CORE:
                               (su0 % GROUP) * TOK_PER_CORE + cols],
                        in_=og[:, 0:cols])

    nc.compile()
    _PROG = nc
    return nc


def _prep_core_input(xs):
    """[1024, 4096] fp16 token-major -> [8, 128, 4096] feature-major tiles.

    xprep[g, p, cc*1024 + t] = xs[t, (4g+cc)*128 + p]
    """
    xt = xs.T.reshape(N_GROUPS, GROUP, P, TOK_PER_CORE)   # [g][cc][p][t]
    return np.ascontiguousarray(
        xt.transpose(0, 2, 1, 3).reshape(N_GROUPS, P, GROUP * TOK_PER_CORE))


def _unprep_core_output(outp):
    """Inverse of _prep_core_input; fp16 device output -> fp32 token-major."""
    o = outp.reshape(N_GROUPS, P, GROUP, TOK_PER_CORE).transpose(0, 2, 1, 3)
    return o.reshape(N, TOK_PER_CORE).T.astype(np.float32)


def kernel(x, factors, bias):
    from concourse.bass_utils import run_bass_kernel_spmd

    x = np.asarray(x, dtype=np.float32)
    factors = np.asarray(factors, dtype=np.float32)
    bias_np = np.asarray(bias, dtype=np.float32)
    assert x.shape == (TOKENS, N)

    m4 = _compose_weights(factors)          # [4, N] float64
    # Masked stationary weights, host-built: for chunk c the 128x128 block
    # W_c[k, j] = (k//4 == j//4) * m4[k%4, c*128+j];  wt[k, c*128+j] = W_c.
    pidx = np.arange(P)
    blk = ((pidx[:, None] // 4) == (pidx[None, :] // 4))      # [128, 128]
    blk_t = np.tile(blk, (1, N_CHUNKS))                       # [128, N]
    wt = np.ascontiguousarray(
        (blk_t * m4[pidx % 4, :]).astype(np.float16))
    biast = np.ascontiguousarray(bias_np.reshape(N_CHUNKS, P).T)

    nc = _get_program()
    x16 = x.astype(np.float16)
    in_maps = []
    for c in range(NCORES):
        in_maps.append({
            "xp": _prep_core_input(
                x16[c * TOK_PER_CORE:(c + 1) * TOK_PER_CORE]),
            "wt": wt,
            "biast": biast,
        })
    res = run_bass_kernel_spmd(nc, in_maps, core_ids=list(range(NCORES)))
    out = np.empty((TOKENS, N), dtype=np.float32)
    for c in range(NCORES):
        out[c * TOK_PER_CORE:(c + 1) * TOK_PER_CORE] = _unprep_core_output(
            res.results[c]["outp"])
    return out



# revision 38
# speedup vs baseline: 1.1095x; 1.0453x over previous
"""ButterflyLinear Trainium2 kernel.

Math insight: every one of the 12 butterfly stages pairs features strictly
within aligned groups of 4 (stage 0 pairs (4k,4k+1),(4k+2,4k+3); stages 1..11
all pair (4k,4k+2),(4k+1,4k+3)).  The whole network therefore collapses
exactly to a block-diagonal linear map with 1024 independent 4x4 blocks:

    out[t, 4k+j] = sum_i x[t, 4k+i] * M_k[i, j] + bias[4k+j]

M is extracted on the host (float64) by pushing the 4 group-basis vectors
through the stage chain.  The device kernel is a feature-major matmul pass:
the host ships x pre-transposed (feature-major tiles, contiguous rows),
each 128-feature chunk is one stationary-weight matmul
out_c[of, tok] = W_c.T @ x_c[if, tok] with N=512 tokens moving, bias added
per-partition during the PSUM->SBUF copy, and the host un-transposes the
returned output.  No on-device transposes, no identity, no bias broadcast.

x and out travel as fp16 (quantization rel err ~9e-4, well inside the 2e-2
gate), halving HBM traffic vs fp32 — this kernel is HBM-bound, so bytes
moved IS the runtime.  Weights are built on-device in fp32 and cast to fp16
for the PE's 2x 16-bit path; PSUM accumulation stays fp32.

Sharding: data-parallel over tokens, 8192/8 = 1024 tokens per core.
"""

import numpy as np

TOKENS = 8192
N = 4096
DEPTH = 12
NCORES = 8
TOK_PER_CORE = TOKENS // NCORES  # 1024
P = 128                  # partitions
N_CHUNKS = N // P        # 32 feature chunks of 128
GROUP = 4                # chunks per x/out group tile (4*1024 tok = 16KB rows)
N_GROUPS = N_CHUNKS // GROUP   # 8
TBLK = 512               # moving-token block per matmul (fp32 N<=512)
N_TBLK = TOK_PER_CORE // TBLK  # 2


def _apply_stage_np(x, factor, stage):
    B, n = x.shape
    block = 1 << (stage + 1)
    half = block >> 1
    m = n // block
    staged = x.reshape(B, m, half, 2).transpose(0, 1, 3, 2)
    pairs = staged.reshape(B, n // 2, 2)
    t = np.einsum("bnc,ncd->bnd", pairs, factor)
    t = t.reshape(B, m, 2, half).transpose(0, 1, 3, 2)
    return t.reshape(B, n)


def _compose_weights(factors):
    """Return M_cols [4, N] float64: M_cols[i, m] = Mfull[4*(m//4)+i, m]."""
    V = np.zeros((4, N), dtype=np.float64)
    for i in range(4):
        V[i, i::4] = 1.0
    M = V
    f64 = np.asarray(factors, dtype=np.float64)
    for s in range(DEPTH):
        M = _apply_stage_np(M, f64[s], s)
    return M


_PROG = None


def _get_program():
    global _PROG
    if _PROG is not None:
        return _PROG

    import concourse.mybir as mybir
    import concourse.tile as tile
    from concourse import bacc

    nc = bacc.Bacc("TRN2", target_bir_lowering=False, debug=False,
                   num_devices=NCORES)
    f32 = mybir.dt.float32
    f16 = mybir.dt.float16
    xp_h = nc.dram_tensor("xp", [N_GROUPS, P, GROUP * TOK_PER_CORE], f16,
                          kind="ExternalInput")
    wt_h = nc.dram_tensor("wt", [P, N], f16, kind="ExternalInput")
    bt_h = nc.dram_tensor("biast", [P, N_CHUNKS], f32, kind="ExternalInput")
    op_h = nc.dram_tensor("outp", [N_GROUPS, P, GROUP * TOK_PER_CORE], f16,
                          kind="ExternalOutput")

    xp = xp_h.ap()
    op = op_h.ap()

    with tile.TileContext(nc) as tc:
        with (
            tc.tile_pool(name="singles", bufs=1) as singles,
            tc.tile_pool(name="xin", bufs=3) as xpool,
            tc.tile_pool(name="oout", bufs=3) as opool,
            tc.tile_pool(name="ps", bufs=4, space="PSUM") as pspool,
        ):
            bias_sb = singles.tile([P, N_CHUNKS], f32)
            nc.gpsimd.dma_start(out=bias_sb, in_=bt_h.ap())
            # Stationary weights come pre-masked from the host (1MB fp16).
            # First half leads the sync ring (it gates the very first
            # matmuls, and every 128-row DMA costs ~2.6us of descriptor
            # generation); second half rides the store ring, which is idle
            # until ~14us.  Putting both on the store ring instead delays
            # the store stream behind the W transfers (FIFO) and measures
            # ~8us slower; SWDGE is ~10x too slow for either.
            w_sb = singles.tile([P, N], f16)
            nc.sync.dma_start(out=w_sb[:, 0:N // 2], in_=wt_h.ap()[:, 0:N // 2])
            nc.scalar.dma_start(out=w_sb[:, N // 2:N], in_=wt_h.ap()[:, N // 2:N])

            # DMA rings retire ~1 descriptor / 20ns regardless of size and
            # descriptors are partition-row-sized, so ring bandwidth is
            # proportional to the contiguous row length.  Whole-group
            # transfers (4 chunks = 8KB fp16 rows, 1MB per DMA) keep both
            # rings HBM-bound instead of descriptor-bound; they also give
            # the PE long uninterrupted matmul runs so HAM warms to the
            # 2.4GHz clock.  Stores taper (2,1,1) at the end to shorten
            # the drain->last-store tail.
            # Group 0 loads in two halves so the first matmuls start one
            # descriptor-generation quantum (~2.8us) earlier.
            load_units = [(0, 2), (2, 2)]
            load_units += [(g * GROUP, GROUP) for g in range(1, N_GROUPS)]
            # Uniform 4-chunk store units (1MB, 8KB rows); the tail tapers
            # 2,1,1 to shorten the last drain->store latency.  Splitting
            # the FIRST store unit as well measures slower: its extra
            # descriptor-generation quantum on the ACT ring delays every
            # following store.
            store_units = [(g * GROUP, GROUP) for g in range(N_GROUPS - 1)]
            store_units += [((N_GROUPS - 1) * GROUP, 2),
                            ((N_GROUPS - 1) * GROUP + 2, 1),
                            ((N_GROUPS - 1) * GROUP + 3, 1)]
            load_at = {c0: n for c0, n in load_units}
            store_of = {}
            for c0, n in store_units:
                for cc in range(n):
                    store_of[c0 + cc] = (c0, n, cc == n - 1)

            xg = og = None
            lu0 = su0 = 0
            for c in range(N_CHUNKS):
                if c in load_at:
                    lu0 = c
                    ln = load_at[c]
                    xg = xpool.tile([P, GROUP * TOK_PER_CORE], f16, tag="xg")
                    nc.sync.dma_start(
                        out=xg[:, 0:ln * TOK_PER_CORE],
                        in_=xp[c // GROUP, :,
                               (c % GROUP) * TOK_PER_CORE:
                               (c % GROUP + ln) * TOK_PER_CORE])
                su0, snch, closes = store_of[c]
                if c == su0:
                    og = opool.tile([P, GROUP * TOK_PER_CORE], f16, tag="og")
                # One 2-bank PSUM tile per chunk: both token-block matmuls
                # land in it, one FD=1024 op drains it.
                ps = pspool.tile([P, TOK_PER_CORE], f32, tag="ps")
                for tb in range(N_TBLK):
                    nc.tensor.matmul(
                        ps[:, tb * TBLK:(tb + 1) * TBLK],
                        lhsT=w_sb[:, c * P:(c + 1) * P],
                        rhs=xg[:, (c - lu0) * TOK_PER_CORE + tb * TBLK:
                               (c - lu0) * TOK_PER_CORE + (tb + 1) * TBLK],
                        start=True, stop=True,
                    )
                bcol = bias_sb[:, c:c + 1]
                # PSUM->SBUF drains run in slow 1x mode (PSUM source), so
                # they are the scarce resource: EVERY chunk's two PSUM
                # banks drain concurrently, ACT taking one and DVE the
                # other, halving per-chunk drain latency.
                o0 = (c - su0) * TOK_PER_CORE
                nc.scalar.add(og[:, o0:o0 + TBLK], ps[:, 0:TBLK], bcol)
                nc.vector.tensor_scalar_add(
                    og[:, o0 + TBLK:o0 + 2 * TBLK], ps[:, TBLK:2 * TBLK],
                    bcol)
                if closes:
                    cols = snch * TOK_PER_CORE
                    nc.scalar.dma_start(
                        out=op[su0 // GROUP, :,
                               (su0 % GROUP) * TOK_PER_CORE:
                               (su0 % GROUP) * TOK_PER_CORE + cols],
                        in_=og[:, 0:cols])

    nc.compile()
    _PROG = nc
    return nc


def _prep_core_input(xs):
    """[1024, 4096] fp16 token-major -> [8, 128, 4096] feature-major tiles.

    xprep[g, p, cc*1024 + t] = xs[t, (4g+cc)*128 + p]
    """
    xt = xs.T.reshape(N_GROUPS, GROUP, P, TOK_PER_CORE)   # [g][cc][p][t]
    return np.ascontiguousarray(
        xt.transpose(0, 2, 1, 3).reshape(N_GROUPS, P, GROUP * TOK_PER_CORE))


def _unprep_core_output(outp):
    """Inverse of _prep_core_input; fp16 device output -> fp32 token-major."""
    o = outp.reshape(N_GROUPS, P, GROUP, TOK_PER_CORE).transpose(0, 2, 1, 3)
    return o.reshape(N, TOK_PER_CORE).T.astype(np.float32)


def kernel(x, factors, bias):
    from concourse.bass_utils import run_bass_kernel_spmd

    x = np.asarray(x, dtype=np.float32)
    factors = np.asarray(factors, dtype=np.float32)
    bias_np = np.asarray(bias, dtype=np.float32)
    assert x.shape == (TOKENS, N)

    m4 = _compose_weights(factors)          # [4, N] float64
    # Masked stationary weights, host-built: for chunk c the 128x128 block
    # W_c[k, j] = (k//4 == j//4) * m4[k%4, c*128+j];  wt[k, c*128+j] = W_c.
    pidx = np.arange(P)
    blk = ((pidx[:, None] // 4) == (pidx[None, :] // 4))      # [128, 128]
    blk_t = np.tile(blk, (1, N_CHUNKS))                       # [128, N]
    wt = np.ascontiguousarray(
        (blk_t * m4[pidx % 4, :]).astype(np.float16))
    biast = np.ascontiguousarray(bias_np.reshape(N_CHUNKS, P).T)

    nc = _get_program()
    x16 = x.astype(np.float16)
    in_maps = []
    for c in range(NCORES):
        in_maps.append({
            "xp": _prep_core_input(
                x16[c * TOK_PER_CORE:(c + 1) * TOK_PER_CORE]),
            "wt": wt,
            "biast": biast,
        })
    res = run_bass_kernel_spmd(nc, in_maps, core_ids=list(range(NCORES)))
    out = np.empty((TOKENS, N), dtype=np.float32)
    for c in range(NCORES):
        out[c * TOK_PER_CORE:(c + 1) * TOK_PER_CORE] = _unprep_core_output(
            res.results[c]["outp"])
    return out

